# revision 16
# baseline (speedup 1.0000x reference)
"""Trainium2 Bass kernel for an 8-head cross-attention block.

Math (per reference):
    Q = video @ Wq[h]           [4096, 64]  per head
    K = text  @ Wk[h]           [1024, 64]
    V = text  @ Wv[h]           [1024, 64]
    att = softmax(Q @ K^T)      [4096, 1024]   (no scaling)
    y_h = att @ V               [4096, 64]
    out = concat_h(y_h) @ Wout + pos_enc(4096, 512)

Sharding: head-parallel over 8 NeuronCores. Core h owns head h and the
matching 64 rows of Wout (row-parallel). The device produces the
UNNORMALIZED projection out_h = (exp(E)@V') @ Wout_h plus the softmax
denominators den_h (per token); since the per-token 1/den scale commutes
with the output projection, the host applies out_h/den_h during the
all-reduce gather (together with the positional encoding).

Device pipeline (single fused loop over 8 chunks of 512 query tokens):
    Qproj(j+1) -> E pairs(j+1) -> exp(j+1)    [PE + ACT]
    PV(j) -> y dup -> out-proj pairs(j) -> cast -> DMA out(j)
All activations stay fp16 ([feature, token] layout, no transposes);
PSUM accumulates fp32. E and out-proj matmuls contract over K=64, so
each pair runs concurrently on the two 64-row PE tiles (T0/T8 row
tiling via base_partition). exp runs as exp(E - 12) on ACT; the shift
cancels in the host-side normalization. The denominator comes free as
a 65th output row of the att@V matmul via a ones-column appended to V.
Elementwise PSUM->SBUF traffic is split across Vector and GpSimd.
"""

import numpy as np

from concourse import bacc
import concourse.mybir as mybir
from concourse.tile import TileContext
from concourse.bass_utils import run_bass_kernel_spmd

N, M, D, H, DH = 4096, 1024, 512, 8, 64
P = 128
NC = 512          # n-chunk width for the attention pipeline
NJ = N // NC      # 8 n-chunks
DC = D // P       # 4 contraction chunks of 128
MT = M // P       # 8 key tiles of 128
F32 = mybir.dt.float32
FP16 = mybir.dt.float16
EXP = mybir.ActivationFunctionType.Exp
EXP_SHIFT = -12.0  # exp(E + shift): keeps exp in fp16 range; cancels in out/den
NCORES = 8

_CACHE: dict = {}
TRACE = False          # test harness can flip this before calling kernel()
LAST_RESULT = None     # BassKernelResults of the last run (for profiling)


def _body(tc, nc, vT, tT, wq, wk, wv, wo, out, den):
    with tc.tile_pool(name="const", bufs=1) as cp, \
         tc.tile_pool(name="pt", bufs=12) as pt_pool, \
         tc.tile_pool(name="ysb", bufs=3) as ysb_pool, \
         tc.tile_pool(name="ot", bufs=2) as ot_pool, \
         tc.tile_pool(name="ps_e", bufs=3, space="PSUM") as e_pool, \
         tc.tile_pool(name="ps_o", bufs=2, space="PSUM") as o_pool:

        vt_sb = cp.tile([P, DC * N], FP16, tag="vt")
        tt_sb = cp.tile([P, DC * M], FP16, tag="tt")
        wq_sb = cp.tile([P, DC * DH], FP16, tag="wq")
        wk_sb = cp.tile([P, DC * DH], FP16, tag="wk")
        wv_sb = cp.tile([P, DC * DH], FP16, tag="wv")
        wo_sb = cp.tile([DH, D], FP16, tag="wo")
        qt_sb = cp.tile([P, N], FP16, tag="qt")      # Q^T duplicated on both halves
        kt_sb = cp.tile([P, M], FP16, tag="kt")      # K^T duplicated on both halves
        v_sb = cp.tile([P, MT * (DH + 1)], FP16, tag="vsb")
        bias_sb = cp.tile([P, 1], F32, tag="bias")
        warm_sb = cp.tile([P, 1], FP16, tag="warm")

        # ---- input loads; vt goes through the gpsimd DMA queue so its
        # transfers run in parallel with the sync queue's tt/weights ----
        vT3 = vT.rearrange("(c p) n -> p c n", p=P)
        vt3 = vt_sb.rearrange("p (c n) -> p c n", n=N)

        def dma_vt(j):
            # one dma per c-chunk: 4 HW rings move 256KB each in parallel
            # (a single ring runs at only ~40 GB/s)
            sl = slice(j * NC, (j + 1) * NC)
            for c in range(DC):
                nc.sync.dma_start(out=vt3[:, c, sl], in_=vT3[:, c, sl])

        nc.sync.dma_start(out=wk_sb.rearrange("p (c e) -> p c e", e=DH),
                          in_=wk.rearrange("(c p) e -> p c e", p=P))
        nc.sync.dma_start(out=wq_sb.rearrange("p (c e) -> p c e", e=DH),
                          in_=wq.rearrange("(c p) e -> p c e", p=P))
        tT3 = tT.rearrange("(c p) m -> p c m", p=P)
        tt3 = tt_sb.rearrange("p (c m) -> p c m", m=M)
        for c in range(DC):
            nc.sync.dma_start(out=tt3[:, c, :], in_=tT3[:, c, :])
        dma_vt(0)
        nc.sync.dma_start(out=wv_sb.rearrange("p (c e) -> p c e", e=DH),
                          in_=wv.rearrange("(c p) e -> p c e", p=P))
        nc.sync.dma_start(out=wo_sb[:, :], in_=wo[:, :])

        v3 = v_sb.rearrange("p (m e) -> p m e", e=DH + 1)  # [128, 8, 65]
        nc.vector.memset(v3[:, :, DH], 1.0)
        nc.vector.memset(bias_sb[:, :], EXP_SHIFT)
        # touch Exp once so the ~1.3us ACT table load hides under the lead-in
        nc.scalar.activation(warm_sb[:, :], bias_sb[:, :], EXP)

        # ---- K^T and V' projections; K runs twice (col-tiled, concurrent)
        # so one full-width cast lands it on both partition halves ----
        for half in range(M // 512):
            ps = o_pool.tile([P, 512], F32, tag="o")
            sl = slice(half * 512, (half + 1) * 512)
            for c in range(DC):
                nc.tensor.matmul(
                    ps[0:DH, :],
                    wk_sb[:, c * DH:(c + 1) * DH],
                    tt_sb[:, c * M + half * 512: c * M + (half + 1) * 512],
                    start=(c == 0), stop=(c == DC - 1),
                    skip_group_check=True)
                nc.tensor.matmul(
                    ps[DH:P, :],
                    wk_sb[:, c * DH:(c + 1) * DH],
                    tt_sb[:, c * M + half * 512: c * M + (half + 1) * 512],
                    start=(c == 0), stop=(c == DC - 1),
                    skip_group_check=True)
            nc.vector.tensor_copy(out=kt_sb[:, sl], in_=ps[:, :])
        def v_proj():
            for mt in range(MT):
                ps = o_pool.tile([P, 512], F32, tag="o")
                for c in range(DC):
                    nc.tensor.matmul(
                        ps[:, 0:DH],
                        tt_sb[:, c * M + mt * P: c * M + (mt + 1) * P],
                        wv_sb[:, c * DH:(c + 1) * DH],
                        start=(c == 0), stop=(c == DC - 1))
                nc.vector.tensor_copy(out=v3[:, mt, 0:DH], in_=ps[:, 0:DH])

        # ---- fused attention + output pipeline over chunks ----
        # Stage skew: Qproj(t) | E+exp(t-1) | PV(t-2) | out-proj+store(t-3).
        # Within an iteration the PE emits E pairs 0-1, then PV, then E pairs
        # 2-3, then Q and out-proj: the ACT engine always has a full e_ps
        # buffer waiting while the PE works on PV/out-proj, and the E pairs
        # 2-3 land exactly when exp frees their PSUM slots. vt chunk loads
        # split across DMA rings (a single ring moves only ~40 GB/s):
        # 4-way for chunk 0 (lead-in critical path), 2-way dispatched three
        # iterations ahead for the rest.
        out_r = out.rearrange("(g p) d -> p g d", p=P)  # [128, 32, 512]
        pts = {}
        ysbs = {}

        def dma_vt2(j):
            sl = slice(j * NC, (j + 1) * NC)
            for g in range(2):
                cs = slice(g * 2, g * 2 + 2)
                nc.sync.dma_start(out=vt3[:, cs, sl], in_=vT3[:, cs, sl])

        def q_stage(uq):
            sl = slice(uq * NC, (uq + 1) * NC)
            qp = o_pool.tile([P, 512], F32, tag="o")
            for c in range(DC):
                nc.tensor.matmul(
                    qp[0:DH, :],
                    wq_sb[:, c * DH:(c + 1) * DH],
                    vt_sb[:, c * N + uq * NC: c * N + (uq + 1) * NC],
                    start=(c == 0), stop=(c == DC - 1),
                    skip_group_check=True)
                nc.tensor.matmul(
                    qp[DH:P, :],
                    wq_sb[:, c * DH:(c + 1) * DH],
                    vt_sb[:, c * N + uq * NC: c * N + (uq + 1) * NC],
                    start=(c == 0), stop=(c == DC - 1),
                    skip_group_check=True)
            nc.vector.tensor_copy(out=qt_sb[:, sl], in_=qp[:, :])

        def e_stage(ue, pairs):
            sl = slice(ue * NC, (ue + 1) * NC)
            for pair in pairs:
                mt = pair * 2
                e_ps = e_pool.tile([P, 1024], F32, tag="e")
                nc.tensor.matmul(
                    e_ps[:, 0:512],
                    kt_sb[0:DH, mt * P:(mt + 1) * P],
                    qt_sb[0:DH, sl],
                    start=True, stop=True)
                nc.tensor.matmul(
                    e_ps[:, 512:1024],
                    kt_sb[DH:P, (mt + 1) * P:(mt + 2) * P],
                    qt_sb[DH:P, sl],
                    start=True, stop=True)
                pt = pt_pool.tile([P, 1024], FP16, tag="p")
                nc.scalar.activation(pt[:, :], e_ps[:, :], EXP, bias=bias_sb[:, :])
                pts.setdefault(ue, []).append(pt)

        def pv_stage(up):
            yt = e_pool.tile([P, 1024], F32, tag="e")
            yp = yt[0:DH + 1, 0:512]
            for mt in range(MT):
                nc.tensor.matmul(
                    yp[:, :],
                    v3[:, mt, :],
                    pts[up][mt // 2][:, (mt % 2) * 512:(mt % 2 + 1) * 512],
                    start=(mt == 0), stop=(mt == MT - 1))
            del pts[up]
            ysb = ysb_pool.tile([DH + 1, 512], FP16, tag="ysb")
            nc.vector.tensor_copy(out=ysb[:, :], in_=yp[:, :])
            nc.sync.dma_start(out=den[:, up * NC:(up + 1) * NC],
                              in_=ysb[DH:DH + 1, :])
            ysbs[up] = ysb

        def out_stage(uo):
            ysb = ysbs.pop(uo)
            ot = ot_pool.tile([P, 4 * D], FP16, tag="o16")
            for nt in range(4):
                po = o_pool.tile([P, 512], F32, tag="o")
                nc.tensor.matmul(
                    po[0:DH, :],
                    ysb[0:DH, nt * P: nt * P + DH],
                    wo_sb[:, :],
                    start=True, stop=True)
                nc.tensor.matmul(
                    po[DH:P, :],
                    ysb[0:DH, nt * P + DH: (nt + 1) * P],
                    wo_sb[:, :],
                    start=True, stop=True)
                nc.vector.tensor_copy(out=ot[:, nt * D:(nt + 1) * D], in_=po[:, :])
            ot3 = ot.rearrange("p (g d) -> p g d", d=D)
            nc.sync.dma_start(
                out=out_r[:, uo * 4:uo * 4 + 2, :], in_=ot3[:, 0:2, :])
            nc.sync.dma_start(
                out=out_r[:, uo * 4 + 2:uo * 4 + 4, :], in_=ot3[:, 2:4, :])

        for t in range(NJ + 3):
            uq, ue, up, uo = t, t - 1, t - 2, t - 3
            if t == 0:
                dma_vt2(1)
                dma_vt2(2)
            if 1 <= uq + 3 < NJ:
                dma_vt2(uq + 3)
            if 0 <= ue < NJ:
                e_stage(ue, (0, 1))
            if 0 <= up < NJ:
                pv_stage(up)
            if 0 <= ue < NJ:
                e_stage(ue, (2, 3))
            if t == 1:
                v_proj()
            if uq < NJ:
                q_stage(uq)
            if 0 <= uo:
                out_stage(uo)

def _build():
    nc = bacc.Bacc("TRN2", target_bir_lowering=False, debug=False)
    vT = nc.dram_tensor("vT", [D, N], FP16, kind="ExternalInput")
    tT = nc.dram_tensor("tT", [D, M], FP16, kind="ExternalInput")
    wq = nc.dram_tensor("wq", [D, DH], FP16, kind="ExternalInput")
    wk = nc.dram_tensor("wk", [D, DH], FP16, kind="ExternalInput")
    wv = nc.dram_tensor("wv", [D, DH], FP16, kind="ExternalInput")
    wo = nc.dram_tensor("wo", [DH, D], FP16, kind="ExternalInput")
    out = nc.dram_tensor("out", [N, D], FP16, kind="ExternalOutput")
    den = nc.dram_tensor("den", [1, N], FP16, kind="ExternalOutput")
    with TileContext(nc) as tc:
        _body(tc, nc, vT[:, :], tT[:, :], wq[:, :], wk[:, :], wv[:, :],
              wo[:, :], out[:, :], den[:, :])
    nc.compile()
    return nc


def _pos_encoding():
    # Mirror the reference's jnp ops bit-for-bit (numpy's f32 sin/exp differ
    # by enough ULPs to dominate the error budget at pos/freq ~ 4e3).
    import jax
    import jax.numpy as jnp
    with jax.default_device(jax.devices("cpu")[0]):
        pos = jnp.arange(N, dtype=jnp.float32)
        freq = jnp.exp(
            (jnp.arange(D // 2, dtype=jnp.float32) / D)
            * jnp.log(jnp.float32(10000.0)))
        x = pos[:, None] / freq
        pe = jnp.stack((jnp.sin(x), jnp.cos(x)), axis=-1)
        return np.asarray(pe.reshape(N, D), dtype=np.float32)


def _fp16(a):
    return np.ascontiguousarray(np.asarray(a, dtype=np.float32).astype(np.float16))


def kernel(video_features, text_features, Wq, Wk, Wv, Wout):
    global LAST_RESULT
    if "nc" not in _CACHE:
        _CACHE["nc"] = _build()
        _CACHE["pe"] = _pos_encoding()
    nc = _CACHE["nc"]

    vT = _fp16(np.asarray(video_features, dtype=np.float32).T)
    tT = _fp16(np.asarray(text_features, dtype=np.float32).T)
    Wq = np.asarray(Wq, dtype=np.float32)
    Wk = np.asarray(Wk, dtype=np.float32)
    Wv = np.asarray(Wv, dtype=np.float32)
    Wout = np.asarray(Wout, dtype=np.float32)

    in_maps = []
    for h in range(NCORES):
        in_maps.append({
            "vT": vT,
            "tT": tT,
            "wq": _fp16(Wq[h]),
            "wk": _fp16(Wk[h]),
            "wv": _fp16(Wv[h]),
            "wo": _fp16(Wout[h * DH:(h + 1) * DH, :]),
        })
    res = run_bass_kernel_spmd(nc, in_maps, list(range(NCORES)), trace=TRACE)
    LAST_RESULT = res
    acc = None
    for h in range(NCORES):
        o = res.results[h]["out"].astype(np.float32)
        d = res.results[h]["den"].astype(np.float32).reshape(N, 1)
        part = o / d
        acc = part if acc is None else acc + part
    return (acc + _CACHE["pe"]).astype(np.float32)
